# revision 1
# baseline (speedup 1.0000x reference)
"""YOLOv1-style loss kernel for Trainium2 (Bass/Tile), data-parallel over 8 cores.

Reference computation (per sample row):
  preds  row: [ pcls: 49*20 | pconf: 49*2 | pbox: 49*2*4 ]  (1470 cols)
  labels row: [ per cell l: obj, tcls[20], tbox[4] ]         (1225 cols)

  o = [pbox.xy/S, pbox.wh^2], t = [tbox.xy/S, tbox.wh]
  iou/rmse best-box select, then
  loss = 0.5*sum(conf parts) + 0.5*obj*(tcls-pcls)^2 + 2.5*obj*(ttgt-pbox[best])^2
  with conf = NOOBJ*pconf^2 everywhere except best box of obj cells where
  OBJ*(best_iou - pconf)^2.  OBJ == NOOBJ == 0.5, so
  conf_total = 0.5*sum(pconf^2) + sum_l 0.5*obj_l*bi_l*(bi_l - 2*pconf_best_l).

Sharding: pure data parallel, batch 16384 -> 8 cores x 2048 rows; each core
produces a scalar partial sum; host adds the 8 partials.
"""

import math

import numpy as np

import concourse.bass as bass
import concourse.bacc as bacc
import concourse.tile as tile
from concourse import mybir
from concourse import bass_utils

S = 7
B = 2
C = 20
L = 49
PC = L * (C + 5 * B)   # 1470
LC = L * (1 + C + 4)   # 1225
P = 128

N_CORES = 8
N_ROWS = 16384
ROWS_PER_CORE = N_ROWS // N_CORES  # 2048

F32 = mybir.dt.float32
Alu = mybir.AluOpType
Act = mybir.ActivationFunctionType


def emit_loss_kernel(nc, tc, preds_h, labels_h, out_h, rows, groups_per_iter,
                     debug_dumps=None, repeat=1, use_gpsimd=True, compute=True):
    """Emit the loss kernel body. rows must be a multiple of 128*groups_per_iter.

    debug_dumps: optional dict name -> DRAM handle; when set, iteration 0's
    intermediate planes are DMA'd out for comparison against a host model.
    """
    G = groups_per_iter
    assert rows % (P * G) == 0
    iters = rows // (P * G)
    n_acc = iters * repeat * 4

    def dump(name, tile_ap):
        if debug_dumps is not None and name in debug_dumps:
            nc.sync.dma_start(out=debug_dumps[name][:], in_=tile_ap)

    preds_d = preds_h[:]
    labels_d = labels_h[:]

    import contextlib
    ctx = contextlib.ExitStack()
    with ctx:
        io_pool = ctx.enter_context(tc.tile_pool(name="io", bufs=2))
        sc = ctx.enter_context(tc.tile_pool(name="scratch", bufs=1))
        sc2 = ctx.enter_context(tc.tile_pool(name="scratch2", bufs=2))
        singles = ctx.enter_context(tc.tile_pool(name="singles", bufs=1))

        acc_big = singles.tile([P, n_acc], F32, tag="acc_big")
        gp = nc.gpsimd if use_gpsimd else nc.vector

        for rawit in range(iters * repeat):
            it = rawit % iters
            r0 = it * P * G

            PT = io_pool.tile([P, G, PC], F32, tag="PT")
            LT = io_pool.tile([P, G, LC], F32, tag="LT")
            nc.sync.dma_start(
                out=PT[:, :, :],
                in_=preds_d[r0 : r0 + P * G, :].rearrange("(g p) c -> p g c", p=P),
            )
            nc.sync.dma_start(
                out=LT[:, :, :],
                in_=labels_d[r0 : r0 + P * G, :].rearrange("(g p) c -> p g c", p=P),
            )

            if not compute:
                nc.vector.tensor_scalar(
                    out=acc_big[:, rawit * 4 : rawit * 4 + 1],
                    in0=PT[:, :, 0:1].rearrange("p g c -> p (g c)")[:, 0:1],
                    scalar1=0.0, scalar2=None, op0=Alu.mult,
                )
                nc.vector.tensor_scalar(
                    out=acc_big[:, rawit * 4 + 1 : rawit * 4 + 2],
                    in0=LT[:, :, 0:1].rearrange("p g c -> p (g c)")[:, 0:1],
                    scalar1=0.0, scalar2=None, op0=Alu.mult,
                )
                nc.vector.memset(acc_big[:, rawit * 4 + 2 : rawit * 4 + 4], 0.0)
                continue

            # ---- input views ----
            pcls = PT[:, :, 0 : L * C].rearrange("p g (l c) -> p g l c", c=C)     # [P,G,49,20]
            pconf = PT[:, :, L * C : L * C + L * B]                               # [P,G,98]
            pconf_lb = pconf.rearrange("p g (l b) -> p g l b", b=B)               # [P,G,49,2]
            pbox_jk = PT[:, :, L * C + L * B :].rearrange("p g (j k) -> p g j k", k=4)
            pbox_lbk = PT[:, :, L * C + L * B :].rearrange(
                "p g (l b k) -> p g l b k", b=B, k=4
            )                                                                      # [P,G,49,2,4]
            LT4 = LT.rearrange("p g (l e) -> p g l e", e=1 + C + 4)               # [P,G,49,25]
            obj = LT4[:, :, :, 0]                                                  # [P,G,49]
            obj1 = LT4[:, :, :, 0:1]                                               # [P,G,49,1]
            tcls = LT4[:, :, :, 1 : 1 + C]                                         # [P,G,49,20]
            tb_xy = LT4[:, :, :, 1 + C : 3 + C]                                    # [P,G,49,2]
            tb_wh = LT4[:, :, :, 3 + C : 5 + C]                                    # [P,G,49,2]

            # ---- transformed predicted boxes o4 = [x/S, y/S, w^2, h^2] ----
            o4 = sc.tile([P, G, L * B * 4], F32, tag="o4")
            o4_jk = o4.rearrange("p g (j k) -> p g j k", k=4)
            o4_lbk = o4.rearrange("p g (l b k) -> p g l b k", b=B, k=4)
            nc.scalar.activation(
                out=o4_jk[:, :, :, 0:2], in_=pbox_jk[:, :, :, 0:2],
                func=Act.Copy, scale=1.0 / S,
            )
            nc.scalar.activation(
                out=o4_jk[:, :, :, 2:4], in_=pbox_jk[:, :, :, 2:4], func=Act.Square
            )

            # transformed truth xy: t4xy = tbox.xy / S   (truth wh is raw tb_wh)
            t4xy = sc.tile([P, G, L * 2], F32, tag="t4xy")
            t4xy_lk = t4xy.rearrange("p g (l k) -> p g l k", k=2)
            nc.scalar.activation(out=t4xy_lk, in_=tb_xy, func=Act.Copy, scale=1.0 / S)
            if it == 0:
                dump("o4", o4[:, :, :])
                dump("t4xy", t4xy[:, :, :])

            # ---- d4 = o - t (per box), interleaved (x,y,w,h) per j ----
            d4 = sc2.tile([P, G, L * B * 4], F32, tag="d4")
            d4_jk = d4.rearrange("p g (j k) -> p g j k", k=4)
            d4_lbk = d4.rearrange("p g (l b k) -> p g l b k", b=B, k=4)
            for b in range(B):
                nc.vector.tensor_sub(
                    d4_lbk[:, :, :, b, 0:2], o4_lbk[:, :, :, b, 0:2], t4xy_lk
                )
                nc.vector.tensor_sub(
                    d4_lbk[:, :, :, b, 2:4], o4_lbk[:, :, :, b, 2:4], tb_wh
                )

            # |center diffs| for the intersection-overlap form
            if it == 0:
                dump("d4", d4[:, :, :])
            adc = sc.tile([P, G, L * B * 4], F32, tag="adc")
            adc_jk = adc.rearrange("p g (j k) -> p g j k", k=4)
            nc.scalar.activation(out=adc, in_=d4[:, :, :], func=Act.Abs)

            # squared diffs (in place) then per-box rmse^2
            nc.scalar.activation(
                out=d4[:, :, :], in_=d4[:, :, :], func=Act.Square
            )
            if it == 0:
                dump("adc", adc[:, :, :])
                dump("sq4", d4[:, :, :])
            # clip = max(|dc|, 0.5*|dw|) per axis per box
            clip = sc.tile([P, G, L * B * 2], F32, tag="clip")
            clip_j2 = clip.rearrange("p g (j k) -> p g j k", k=2)
            nc.vector.scalar_tensor_tensor(
                out=clip_j2, in0=adc_jk[:, :, :, 2:4], scalar=0.5,
                in1=adc_jk[:, :, :, 0:2], op0=Alu.mult, op1=Alu.max,
            )
            ssb = sc.tile([P, G, L * B], F32, tag="ssb")
            ssb_lb = ssb.rearrange("p g (l b) -> p g l b", b=B)
            nc.vector.reduce_sum(out=ssb, in_=d4_jk, axis=mybir.AxisListType.X)

            if it == 0:
                dump("ssb", ssb[:, :, :])
            # overlap per axis: ov = 0.5*(o.wh + t.wh) - |dc| ; relu; inter = ovx*ovy
            n1 = sc.tile([P, G, L * B * 2], F32, tag="n1")
            n1_lbk = n1.rearrange("p g (l b k) -> p g l b k", b=B, k=2)
            n1_j2 = n1.rearrange("p g (j k) -> p g j k", k=2)
            for b in range(B):
                nc.vector.tensor_add(
                    n1_lbk[:, :, :, b, :], o4_lbk[:, :, :, b, 2:4], tb_wh
                )
            nc.vector.scalar_tensor_tensor(
                out=n1[:, :, :], in0=n1[:, :, :], scalar=0.5, in1=clip[:, :, :],
                op0=Alu.mult, op1=Alu.subtract,
            )
            nc.scalar.activation(out=n1[:, :, :], in_=n1[:, :, :], func=Act.Relu)
            if it == 0:
                dump("ovr", n1[:, :, :])
            inter = sc.tile([P, G, L * B], F32, tag="inter")
            inter_lb = inter.rearrange("p g (l b) -> p g l b", b=B)
            nc.vector.tensor_mul(inter, n1_j2[:, :, :, 0], n1_j2[:, :, :, 1])

            if it == 0:
                dump("inter", inter[:, :, :])
            # areas and union
            oA = sc.tile([P, G, L * B], F32, tag="oA")
            oA_lb = oA.rearrange("p g (l b) -> p g l b", b=B)
            gp.tensor_mul(oA, o4_jk[:, :, :, 2], o4_jk[:, :, :, 3])
            tA = sc.tile([P, G, L], F32, tag="tA")
            gp.tensor_mul(tA, LT4[:, :, :, 3 + C], LT4[:, :, :, 4 + C])
            gp.tensor_scalar_max(tA, tA, 1e-12)
            u1 = sc.tile([P, G, L * B], F32, tag="u1")
            u1_lb = u1.rearrange("p g (l b) -> p g l b", b=B)
            nc.vector.tensor_add(
                u1_lb, oA_lb, tA.unsqueeze(3).broadcast_to((P, G, L, B))
            )
            nc.vector.tensor_sub(u1, u1, inter)  # union (>= 1e-12)

            if it == 0:
                dump("tA", tA[:, :, :])
                dump("oA", oA[:, :, :])
                dump("union", u1[:, :, :])
            rec = sc.tile([P, G, L * B], F32, tag="rec")
            nc.vector.reciprocal_approx_fast(out=rec, in_=u1)
            # iou, stored over inter
            nc.vector.tensor_mul(inter, inter, rec)

            if it == 0:
                dump("iou", inter[:, :, :])
            # ---- best-box select: s = 1 if box1 wins ----
            cgt = sc.tile([P, G, L], F32, tag="cgt")
            nc.vector.tensor_tensor(
                cgt, inter_lb[:, :, :, 1], inter_lb[:, :, :, 0], op=Alu.is_gt
            )
            mx = sc.tile([P, G, L], F32, tag="mx")
            nc.vector.tensor_max(mx, inter_lb[:, :, :, 0], inter_lb[:, :, :, 1])
            # nam = -(mx > 0)
            nc.vector.tensor_scalar(
                out=mx, in0=mx, scalar1=0.0, scalar2=-1.0,
                op0=Alu.is_gt, op1=Alu.mult,
            )
            clt = sc.tile([P, G, L], F32, tag="clt")
            nc.vector.tensor_tensor(
                clt, ssb_lb[:, :, :, 1], ssb_lb[:, :, :, 0], op=Alu.is_lt
            )
            w1 = sc.tile([P, G, L], F32, tag="w1")
            nc.vector.scalar_tensor_tensor(
                out=w1, in0=mx, scalar=1.0, in1=clt, op0=Alu.add, op1=Alu.mult
            )
            nc.vector.tensor_add(w1, w1, cgt)  # w1 := s

            if it == 0:
                dump("s", w1[:, :, :])
                dump("cgt", cgt[:, :, :])
                dump("clt", clt[:, :, :])
            # ---- confidence objective term ----
            # z = iou - 2*pconf ; gg = iou*z ; gb = gg0 + s*(gg1-gg0)
            z = sc.tile([P, G, L * B], F32, tag="z")
            z_lb = z.rearrange("p g (l b) -> p g l b", b=B)
            nc.vector.scalar_tensor_tensor(
                out=z, in0=pconf, scalar=-2.0, in1=inter, op0=Alu.mult, op1=Alu.add
            )
            nc.vector.tensor_mul(z, z, inter)
            dg = sc.tile([P, G, L], F32, tag="dg")
            nc.vector.tensor_sub(dg, z_lb[:, :, :, 1], z_lb[:, :, :, 0])
            nc.vector.tensor_mul(dg, w1, dg)
            nc.vector.tensor_add(dg, z_lb[:, :, :, 0], dg)  # dg := g_best
            if it == 0:
                dump("gb", dg[:, :, :])
            ttr_dump = sc.tile([P, G, L], F32, tag="ttr_dump")
            # out = (gb * 0.5) * obj ; accum = sum(out)   (TTR faults on HW)
            nc.vector.scalar_tensor_tensor(
                out=ttr_dump, in0=dg, scalar=0.5, in1=obj,
                op0=Alu.mult, op1=Alu.mult,
                accum_out=acc_big[:, rawit * 4 : rawit * 4 + 1],
            )

            # sum(0.5 * pconf^2), dumped over rec (dead)
            nc.scalar.activation(
                out=rec, in_=pconf, func=Act.Square, scale=math.sqrt(0.5),
                accum_out=acc_big[:, rawit * 4 + 1 : rawit * 4 + 2],
            )

            # ---- coord term ----
            dd = sc2.tile([P, G, L * 4], F32, tag="dd")
            dd_lk = dd.rearrange("p g (l k) -> p g l k", k=4)
            gp.tensor_sub(
                dd_lk, pbox_lbk[:, :, :, 1, :], pbox_lbk[:, :, :, 0, :]
            )
            gp.tensor_mul(
                dd_lk,
                w1.unsqueeze(3).broadcast_to((P, G, L, 4)),
                dd_lk,
            )
            gp.tensor_add(dd_lk, pbox_lbk[:, :, :, 0, :], dd_lk)  # dd := pbest

            if it == 0:
                dump("pbest", dd[:, :, :])
            ttwh = sc.tile([P, G, L * 2], F32, tag="ttwh")
            ttwh_lk = ttwh.rearrange("p g (l k) -> p g l k", k=2)
            nc.scalar.activation(out=ttwh_lk, in_=tb_wh, func=Act.Sqrt)

            cd = sc2.tile([P, G, L * 4], F32, tag="cd")
            cd_lk = cd.rearrange("p g (l k) -> p g l k", k=4)
            nc.vector.tensor_sub(cd_lk[:, :, :, 0:2], tb_xy, dd_lk[:, :, :, 0:2])
            nc.vector.tensor_sub(cd_lk[:, :, :, 2:4], ttwh_lk, dd_lk[:, :, :, 2:4])
            nc.vector.tensor_mul(
                cd_lk, obj1.broadcast_to((P, G, L, 4)), cd_lk
            )
            if it == 0:
                dump("cdm", cd[:, :, :])
            nc.scalar.activation(
                out=cd[:, :, :], in_=cd[:, :, :], func=Act.Square,
                scale=math.sqrt(2.5),
                accum_out=acc_big[:, rawit * 4 + 2 : rawit * 4 + 3],
            )

            # ---- class term ----
            dcls = sc2.tile([P, G, L * C], F32, tag="dcls")
            dcls_lc = dcls.rearrange("p g (l c) -> p g l c", c=C)
            nc.vector.tensor_sub(dcls_lc, tcls, pcls)
            gp.tensor_mul(
                dcls_lc, obj1.broadcast_to((P, G, L, C)), dcls_lc
            )
            if it == 0:
                dump("dclsm", dcls[:, :, :])
            nc.scalar.activation(
                out=dcls[:, :, :], in_=dcls[:, :, :], func=Act.Square,
                scale=math.sqrt(0.5),
                accum_out=acc_big[:, rawit * 4 + 3 : rawit * 4 + 4],
            )

        # ---- combine partial accumulators and reduce across partitions ----
        total = singles.tile([P, 1], F32, tag="total")
        nc.vector.reduce_sum(out=total, in_=acc_big[:, :], axis=mybir.AxisListType.X)
        ones = singles.tile([P, 1], F32, tag="ones")
        nc.vector.memset(ones, 1.0)
        psum_pool = ctx.enter_context(tc.tile_pool(name="ps", bufs=1, space="PSUM"))
        ps_out = psum_pool.tile([1, 1], F32)
        nc.tensor.matmul(out=ps_out[:, :], lhsT=total[:, :], rhs=ones[:, :],
                         start=True, stop=True)
        final_sb = singles.tile([1, 1], F32, tag="final_sb")
        nc.vector.tensor_copy(out=final_sb[:, :], in_=ps_out[:, :])
        nc.sync.dma_start(out=out_h[:], in_=final_sb[:, :])


def build_nc(rows=ROWS_PER_CORE, groups_per_iter=4, repeat=1, use_gpsimd=True,
             compute=True):
    nc = bacc.Bacc()
    preds_h = nc.dram_tensor("preds", [rows, PC], F32, kind="ExternalInput")
    labels_h = nc.dram_tensor("labels", [rows, LC], F32, kind="ExternalInput")
    out_h = nc.dram_tensor("out", [1, 1], F32, kind="ExternalOutput")
    with tile.TileContext(nc) as tc:
        emit_loss_kernel(nc, tc, preds_h, labels_h, out_h, rows, groups_per_iter,
                         repeat=repeat, use_gpsimd=use_gpsimd, compute=compute)
    nc.compile()
    return nc


_NC_CACHE = {}


def _get_nc(rows, groups_per_iter, repeat=1, use_gpsimd=True, compute=True):
    key = (rows, groups_per_iter, repeat, use_gpsimd, compute)
    if key not in _NC_CACHE:
        _NC_CACHE[key] = build_nc(rows, groups_per_iter, repeat, use_gpsimd, compute)
    return _NC_CACHE[key]


def kernel(preds: np.ndarray, labels: np.ndarray) -> np.ndarray:
    preds = np.ascontiguousarray(preds, dtype=np.float32)
    labels = np.ascontiguousarray(labels, dtype=np.float32)
    n = preds.shape[0]
    rows = n // N_CORES
    nc = _get_nc(rows, 4)
    ps = preds.reshape(N_CORES, rows, PC)
    ls = labels.reshape(N_CORES, rows, LC)
    in_maps = [{"preds": ps[i], "labels": ls[i]} for i in range(N_CORES)]
    res = bass_utils.run_bass_kernel_spmd(nc, in_maps, core_ids=list(range(N_CORES)))
    total = sum(float(r["out"][0, 0]) for r in res.results)
    return np.float32(total)

